# revision 28
# baseline (speedup 1.0000x reference)
"""AnalyticHashLinear Trainium2 kernel (8 NeuronCores, batch-sharded SPMD).

y = x @ W^T + bias,  W[o,i] = cb[(c0 + 10007*o + 20011*i) mod 2^16] * (-1)^(o+i+1)

Key algebra (all mod 2^16):
  inv(20011) = 131, 131*10007 = 197, c0 = 3*40009 mod 2^16 = 54491, t0 = 131*c0 = 60433
  P1[j] := cb[20011*j]  =>  W[o,i] = P1[t0 + 197*o + i]
  signs: (-1)^(o+i) = (-1)^(j - t0)  (197 odd) -> folded into table P1s.
  Table build uses the run structure P1[j0 + 131*k] = cb[20011*j0 + k] (contiguous
  codebook runs): a few chunky DMAs + 8 xbar transposes, no per-element moves.
  W-natural tiles [o-part, i-free] are single big window DMAs from P1s_rep
  (partition stride 197, free contiguous); the [o,i]->[i,o] flip happens on the
  TensorEngine (transpose-matmul), avoiding thousands of small xbar DMAs whose
  issue cost dominated.

Each core: 1024 rows of x, full out_dim. No collectives.
"""
import os
import numpy as np
import ml_dtypes

import concourse.bass as bass
import concourse.tile as tile
import concourse.mybir as mybir

F32 = mybir.dt.float32
BF16 = mybir.dt.bfloat16
nbf16 = ml_dtypes.bfloat16

MOD = 65536
HA, HB, HC = 10007, 20011, 40009
LAYER = 3
C0 = (LAYER * HC) % MOD            # 54491
U = pow(HB, -1, MOD)               # 131
S = (U * HA) % MOD                 # 197
T0 = (U * C0) % MOD                # 60433

NB = 1024                          # batch rows per core
IN_DIM = 4096
OUT_DIM = 4096
KT = IN_DIM // 128                 # 32 contraction tiles
KTX = 0                            # kt < KTX use xbar transpose; rest PE
WNF = 21 * S                       # 4137: wnat free (covers (KT-KTX)*128 + slack)
TBL = 95232                        # P1s_rep length (>= 65535 + 127*197 + WNF)
CBREP = 2625536                    # cb16 replicated length
OC = 1024                          # o-chunk
NOC = OUT_DIM // OC                # 4
WT_BUFS = 66


def _consts():
    # eps on T1 [j0-part, k-free]: +1 iff (j0 + k) odd
    p = np.arange(128)[:, None]
    f = np.arange(512)[None, :]
    eps1 = np.where(((p + f) % 2) == 1, 1.0, -1.0).astype(nbf16)
    # eps on T2 (j0 = 115 + p): +1 iff (p + f) even
    p2 = np.arange(16)[:, None]
    eps2 = np.where(((p2 + f) % 2) == 0, 1.0, -1.0).astype(nbf16)
    ident = np.eye(128, dtype=nbf16)
    ones = np.ones((1, 128), dtype=nbf16)
    return eps1, eps2, ident, ones


def _win_ap(p1_h, base):
    """[128, 4137] overlapping-window view: addr(p, f) = base + 197*p + f —
    per-partition WNF contiguous elements, partition stride 197 (overlap is
    fine for reads; built via AP surgery since rearrange can't overlap)."""
    a = p1_h[base:base + S * 128].rearrange("(p s) -> p s", s=S).copy()
    a.ap = type(a.ap)([[S, 128], [1, WNF]])
    return a


def build_kernel(tc: tile.TileContext, out_h, x_h, cb_h, bias_h):
    nc = tc.nc
    eps1_np, eps2_np, ident_np, ones_np = _consts()
    eps1_h = nc.inline_tensor(eps1_np, name="eps1")
    eps2_h = nc.inline_tensor(eps2_np, name="eps2")
    ident_h = nc.inline_tensor(ident_np, name="ident")
    ones_h = nc.inline_tensor(ones_np, name="ones1")

    cbrep_h = nc.dram_tensor("cbrep", [CBREP], BF16, kind="Internal")
    p1_h = nc.dram_tensor("p1rep", [TBL], BF16, kind="Internal")
    xb_h = nc.dram_tensor("xb16", [NB, IN_DIM], BF16, kind="Internal")

    from contextlib import ExitStack
    with ExitStack() as ctx:
        build_pool = ctx.enter_context(tc.tile_pool(name="build", bufs=1))
        const_pool = ctx.enter_context(tc.tile_pool(name="const", bufs=1))
        xt_pool = ctx.enter_context(tc.tile_pool(name="xt", bufs=1))
        wn_pool = ctx.enter_context(tc.tile_pool(name="wn", bufs=2))
        wt_pool = ctx.enter_context(tc.tile_pool(name="wt", bufs=WT_BUFS))
        y_pool = ctx.enter_context(tc.tile_pool(name="y", bufs=3))
        ps_pool = ctx.enter_context(tc.tile_pool(name="ps", bufs=4, space="PSUM"))
        pt_pool = ctx.enter_context(tc.tile_pool(name="pt", bufs=4, space="PSUM"))

        # ---- gpsimd cast DMAs (f32 -> bf16) ----
        nc.gpsimd.dma_start(cbrep_h[0:MOD], cb_h[:])
        for bt in range(NB // 128):
            nc.gpsimd.dma_start(
                xb_h[bt * 128:(bt + 1) * 128, :].flatten(),
                x_h[bt * 128:(bt + 1) * 128, :].flatten(),
            )
        bias16 = const_pool.tile([1, OUT_DIM], BF16)
        nc.gpsimd.dma_start(bias16[:], bias_h[:])       # cast f32 -> bf16

        # ---- constants ----
        ident_sb = const_pool.tile([128, 128], BF16)
        nc.sync.dma_start(ident_sb[:], ident_h[:, :])
        ones_sb = const_pool.tile([1, 128], BF16)
        nc.sync.dma_start(ones_sb[:], ones_h[:, :])

        # ---- x^T via xbar transposes (single HWDGE master: nc.sync) ----
        xT = xt_pool.tile([128, KT * NB], BF16)
        def emit_xt():
            for bt in range(NB // 128):
                for kt in range(KT):
                    nc.sync.dma_start(
                        xT[:, kt * NB + bt * 128: kt * NB + (bt + 1) * 128],
                        xb_h[bt * 128:(bt + 1) * 128, kt * 128:(kt + 1) * 128],
                        transpose=True,
                    )

        # ---- table build (parallel replication: all copies read [0:MOD]) ----
        n = MOD
        while n < CBREP:
            m = min(MOD, CBREP - n)
            nc.sync.dma_start(cbrep_h[n:n + m], cbrep_h[0:m])
            n += m

        T1 = build_pool.tile([128, 512], BF16, tag="T1")
        nc.sync.dma_start(
            T1[:],
            cbrep_h[0:128 * HB].rearrange("(p s) -> p s", s=HB)[:, 0:512],
        )
        T2 = build_pool.tile([16, 512], BF16, tag="T2")
        b2 = 115 * HB
        nc.sync.dma_start(
            T2[:],
            cbrep_h[b2:b2 + 16 * HB].rearrange("(p s) -> p s", s=HB)[:, 0:512],
        )
        eps1_sb = build_pool.tile([128, 512], BF16, tag="e1")
        nc.sync.dma_start(eps1_sb[:], eps1_h[:, :])
        eps2_sb = build_pool.tile([16, 512], BF16, tag="e2")
        nc.sync.dma_start(eps2_sb[:], eps2_h[:, :])
        V1 = build_pool.tile([128, 512], BF16, tag="V1")
        nc.vector.tensor_mul(V1[:], T1[:], eps1_sb[:])
        V2 = build_pool.tile([16, 512], BF16, tag="V2")
        nc.vector.tensor_mul(V2[:], T2[:], eps2_sb[:])

        for c in range(4):
            U1 = build_pool.tile([128, 128], BF16, tag="U1")
            nc.sync.dma_start(U1[:], V1[:, 128 * c:128 * (c + 1)], transpose=True)
            off = 131 * 128 * c
            nc.sync.dma_start(
                p1_h[off:off + 131 * 128].rearrange("(p s) -> p s", s=131)[:, 0:128],
                U1[:],
            )
            U2 = build_pool.tile([128, 16], BF16, tag="U2")
            nc.sync.dma_start(U2[:], V2[:, 128 * c:128 * (c + 1)], transpose=True)
            off2 = 115 + 131 * 128 * c
            nc.sync.dma_start(
                p1_h[off2:off2 + 131 * 128].rearrange("(p s) -> p s", s=131)[:, 0:16],
                U2[:],
            )
        # tail replicate (positions >= 65536 mirror j - 65536)
        nc.sync.dma_start(p1_h[MOD:TBL], p1_h[0:TBL - MOD])
        emit_xt()

        # ---- main loop ----
        for oc in range(NOC):
            # W^T tiles for this o-chunk: load W-natural windows (one 1 MB DMA
            # per 128 o's), flip on the PE, drain into wt via DVE/ACT copies.
            wt = []
            for _kt in range(KT):
                wh = []
                for _h in range(2):
                    w = wt_pool.tile([128, 512], BF16, tag="wt",
                                     name=f"wt{_kt}_{_h}")
                    wh.append(w)
                wt.append(wh)
            for osub in range(OC // 128):
                o0 = oc * OC + osub * 128
                base = (T0 + S * o0) % MOD
                # kt in [0, KTX): xbar transpose straight into wt
                for kt in range(KTX):
                    win = p1_h[base + kt * 128:base + kt * 128 + S * 128].rearrange(
                        "(p s) -> p s", s=S
                    )[:, 0:128]
                    nc.sync.dma_start(
                        wt[kt][osub // 4][:, (osub % 4) * 128:(osub % 4 + 1) * 128],
                        win, transpose=True
                    )
                # kt in [KTX, KT): PE transpose from a W-natural window
                wn = wn_pool.tile([128, WNF], BF16, tag="wn")
                nc.sync.dma_start(wn[:], _win_ap(p1_h, base + KTX * 128))
                for kt in range(KTX, KT):
                    pst = pt_pool.tile([128, 128], BF16, tag="pt")
                    nc.tensor.transpose(
                        pst[:], wn[:, (kt - KTX) * 128:(kt - KTX + 1) * 128],
                        ident_sb[:],
                    )
                    dst = wt[kt][osub // 4][:, (osub % 4) * 128:(osub % 4 + 1) * 128]
                    if kt % 2 == 0:
                        nc.vector.tensor_copy(dst, pst[:])
                    else:
                        nc.scalar.copy(dst, pst[:])
            for bt in range(NB // 128):
                ps0 = ps_pool.tile([128, 512], F32, tag="ps")
                ps1 = ps_pool.tile([128, 512], F32, tag="ps")
                for kt in range(KT):
                    xoff = kt * NB + bt * 128
                    lhsT = xT[:, xoff:xoff + 128]
                    nc.tensor.matmul(
                        ps0[:], lhsT, wt[kt][0][:],
                        start=(kt == 0), stop=False,
                    )
                    nc.tensor.matmul(
                        ps1[:], lhsT, wt[kt][1][:],
                        start=(kt == 0), stop=False,
                    )
                for h, ps in ((0, ps0), (1, ps1)):
                    ob = oc * OC + h * 512
                    # bias via K=1 accumulation (closes the psum group)
                    nc.tensor.matmul(
                        ps[:], ones_sb[:], bias16[0:1, ob:ob + 512],
                        start=False, stop=True,
                    )
                    yt = y_pool.tile([128, 512], F32, tag="y")
                    nc.vector.tensor_copy(yt[:], ps[:])
                    nc.sync.dma_start(
                        out_h[bt * 128:(bt + 1) * 128, ob:ob + 512], yt[:]
                    )


# This container's walrus rejects the EVENT_SEMAPHORE_RANGE_CLEAR ISA encoding
# ("ISA wrong length") that TileContext emits when freeing semaphores at kernel
# exit. The preamble zeroes all semaphore banks via InstMemset at the start of
# every execution, so the exit-time clear is redundant — skip emitting it but
# keep the allocator bookkeeping.
def _patched_clear_and_free_semaphores(self, sems):
    if not sems:
        return
    sem_nums = [
        sem.num if isinstance(sem, bass.SemaphoreHandle) else sem for sem in sems
    ]
    self._state.prepend_free_semaphores(sem_nums)
    for poison_set in self._tile_sem_poison_stack:
        poison_set.update(sem_nums)


bass.Bass.clear_and_free_semaphores = _patched_clear_and_free_semaphores


# Same walrus also only encodes ONE sync-wait on non-EventSemaphore
# instructions ("Too many sync wait commands"), but TileContext's kernel-tail
# drain gets one wait per outstanding DMA sem lane. Split the extras onto
# additional drains (sequential on the same engine => same semantics).
def _patched_drain_and_barrier(self, tick_clock, wait_clock):
    import bass_rust as _br
    from concourse.vector_clock import ScopedClock

    nc = self.nc
    drain_inst = nc.sync.drain()
    wait_clock.add_sem_waits(
        drain_inst.ins, ScopedClock({None: tick_clock.global_clock})
    )
    si = drain_inst.ins.sync_info
    if si is not None and si.on_wait and len(si.on_wait) > 1:
        waits = list(si.on_wait)
        si.on_wait = waits[:1]
        for w in waits[1:]:
            d2 = nc.sync.drain()
            d2.ins.sync_info = _br.SyncInfo(on_wait=[w], on_update=[])
    nc.all_engine_barrier()
    assert self.sems is not None
    popped = nc._tile_sem_poison_stack.pop()
    assert popped is self._sem_poison
    nc.clear_and_free_semaphores(list(self.sems.allocated().values()))
    nc.all_engine_barrier()


tile.TileContext._drain_and_barrier = _patched_drain_and_barrier


def _split_multiwait(nc):
    """Walrus in this container encodes at most 1 sync-wait per instruction
    (2 for EventSemaphore). Tile's scheduler attaches more. Move extra waits
    onto InstNoOp carriers inserted just before the instruction in its block
    (same engine => executes in order => identical semantics)."""
    import bass_rust as _br

    for f in nc.m.functions:
        for blk in f.blocks:
            insts = blk.instructions
            i = 0
            while i < len(insts):
                inst = insts[i]
                si = getattr(inst, "sync_info", None)
                cap = 2 if type(inst).__name__ == "InstEventSemaphore" else 1
                if si is not None and si.on_wait and len(si.on_wait) > cap:
                    waits = list(si.on_wait)
                    si.on_wait = waits[:cap]
                    for w in waits[cap:]:
                        nop = nc.engines[inst.engine].nop()
                        nopi = nop.ins
                        nopi.sync_info = _br.SyncInfo(on_wait=[w], on_update=[])
                        src_list = nc.cur_bb.bb.instructions
                        assert src_list[len(src_list) - 1].name == nopi.name
                        src_list.pop()
                        insts.insert(i, nopi)
                        i += 1
                i += 1


def _dedup_ldweights(nc):
    """bass lowers every matmul to Ldweights+Matmult; consecutive matmuls that
    share the stationary operand reload it redundantly (walrus --enable-ldw-opt
    is off in this harness). Drop an InstLdweights when the previous PE
    Ldweights in the block has an identical weights AP and only Matmult/NoOp
    sit between (they don't clobber the array). LDWs carry no sync_info here
    (waits ride on the Matmults), so deletion is semaphore-safe."""
    n_del = 0
    for f in nc.m.functions:
        for blk in f.blocks:
            insts = blk.instructions
            last_key = None
            i = 0
            while i < len(insts):
                inst = insts[i]
                tn = type(inst).__name__
                eng = getattr(inst, "engine", None)
                if eng is not None and str(eng) == "EngineType.PE":
                    if tn == "InstLdweights":
                        si = getattr(inst, "sync_info", None)
                        key = repr(inst.ins[0])
                        if (key == last_key and
                                (si is None or not si.on_wait)):
                            insts.pop(i)
                            n_del += 1
                            continue
                        last_key = key
                    elif tn not in ("InstMatmult", "InstNoOp"):
                        last_key = None
                i += 1
    return n_del


_NC_CACHE = None


def _build_nc():
    global _NC_CACHE
    if _NC_CACHE is not None:
        return _NC_CACHE
    nc = bass.Bass(trn_type="TRN2")
    x_h = nc.dram_tensor("x", [NB, IN_DIM], F32, kind="ExternalInput")
    cb_h = nc.dram_tensor("codebook", [MOD], F32, kind="ExternalInput")
    bias_h = nc.dram_tensor("bias", [OUT_DIM], F32, kind="ExternalInput")
    out_h = nc.dram_tensor("out", [NB, OUT_DIM], F32, kind="ExternalOutput")
    with tile.TileContext(nc) as tc:
        build_kernel(tc, out_h, x_h, cb_h, bias_h)
    _split_multiwait(nc)
    _dedup_ldweights(nc)
    _NC_CACHE = nc
    return nc


def kernel(x, codebook, bias):
    x = np.ascontiguousarray(np.asarray(x, dtype=np.float32))
    codebook = np.ascontiguousarray(np.asarray(codebook, dtype=np.float32))
    bias = np.ascontiguousarray(np.asarray(bias, dtype=np.float32))
    assert x.shape == (8192, 4096)

    from concourse.bass_utils import run_bass_kernel_spmd

    nc = _build_nc()
    in_maps = [
        {"x": x[c * NB:(c + 1) * NB], "codebook": codebook, "bias": bias}
        for c in range(8)
    ]
    trace = os.environ.get("KERNEL_TRACE", "0") == "1"
    res = run_bass_kernel_spmd(nc, in_maps, core_ids=list(range(8)), trace=trace)
    if trace and res.exec_time_ns is not None:
        print(f"HW exec time: {res.exec_time_ns} ns")
    out = np.concatenate([r["out"] for r in res.results], axis=0)
    return out


# revision 29
# speedup vs baseline: 1.0153x; 1.0153x over previous
"""AnalyticHashLinear Trainium2 kernel (8 NeuronCores, batch-sharded SPMD).

y = x @ W^T + bias,  W[o,i] = cb[(c0 + 10007*o + 20011*i) mod 2^16] * (-1)^(o+i+1)

Key algebra (all mod 2^16):
  inv(20011) = 131, 131*10007 = 197, c0 = 3*40009 mod 2^16 = 54491, t0 = 131*c0 = 60433
  P1[j] := cb[20011*j]  =>  W[o,i] = P1[t0 + 197*o + i]
  signs: (-1)^(o+i) = (-1)^(j - t0)  (197 odd) -> folded into table P1s.
  Table build uses the run structure P1[j0 + 131*k] = cb[20011*j0 + k] (contiguous
  codebook runs): a few chunky DMAs + 8 xbar transposes, no per-element moves.
  W-natural tiles [o-part, i-free] are single big window DMAs from P1s_rep
  (partition stride 197, free contiguous); the [o,i]->[i,o] flip happens on the
  TensorEngine (transpose-matmul), avoiding thousands of small xbar DMAs whose
  issue cost dominated.

Each core: 1024 rows of x, full out_dim. No collectives.
"""
import os
import numpy as np
import ml_dtypes

import concourse.bass as bass
import concourse.tile as tile
import concourse.mybir as mybir

F32 = mybir.dt.float32
BF16 = mybir.dt.bfloat16
nbf16 = ml_dtypes.bfloat16

MOD = 65536
HA, HB, HC = 10007, 20011, 40009
LAYER = 3
C0 = (LAYER * HC) % MOD            # 54491
U = pow(HB, -1, MOD)               # 131
S = (U * HA) % MOD                 # 197
T0 = (U * C0) % MOD                # 60433

NB = 1024                          # batch rows per core
IN_DIM = 4096
OUT_DIM = 4096
KT = IN_DIM // 128                 # 32 contraction tiles
KTX = 0                            # kt < KTX use xbar transpose; rest PE
WNF = 21 * S                       # 4137: wnat free (covers (KT-KTX)*128 + slack)
TBL = 95232                        # P1s_rep length (>= 65535 + 127*197 + WNF)
CBREP = 2625536                    # cb16 replicated length
OC = 1024                          # o-chunk
NOC = OUT_DIM // OC                # 4
WT_BUFS = 66


def _consts():
    # eps on T1 [j0-part, k-free]: +1 iff (j0 + k) odd
    p = np.arange(128)[:, None]
    f = np.arange(512)[None, :]
    eps1 = np.where(((p + f) % 2) == 1, 1.0, -1.0).astype(nbf16)
    # eps on T2 (j0 = 115 + p): +1 iff (p + f) even
    p2 = np.arange(16)[:, None]
    eps2 = np.where(((p2 + f) % 2) == 0, 1.0, -1.0).astype(nbf16)
    ident = np.eye(128, dtype=nbf16)
    ones = np.ones((1, 128), dtype=nbf16)
    return eps1, eps2, ident, ones


def _win_ap(p1_h, base):
    """[128, 4137] overlapping-window view: addr(p, f) = base + 197*p + f —
    per-partition WNF contiguous elements, partition stride 197 (overlap is
    fine for reads; built via AP surgery since rearrange can't overlap)."""
    a = p1_h[base:base + S * 128].rearrange("(p s) -> p s", s=S).copy()
    a.ap = type(a.ap)([[S, 128], [1, WNF]])
    return a


def build_kernel(tc: tile.TileContext, out_h, x_h, cb_h, bias_h):
    nc = tc.nc
    eps1_np, eps2_np, ident_np, ones_np = _consts()
    eps1_h = nc.inline_tensor(eps1_np, name="eps1")
    eps2_h = nc.inline_tensor(eps2_np, name="eps2")
    ident_h = nc.inline_tensor(ident_np, name="ident")
    ones_h = nc.inline_tensor(ones_np, name="ones1")

    cbrep_h = nc.dram_tensor("cbrep", [CBREP], BF16, kind="Internal")
    p1_h = nc.dram_tensor("p1rep", [TBL], BF16, kind="Internal")
    xb_h = nc.dram_tensor("xb16", [NB, IN_DIM], BF16, kind="Internal")

    from contextlib import ExitStack
    with ExitStack() as ctx:
        build_pool = ctx.enter_context(tc.tile_pool(name="build", bufs=1))
        const_pool = ctx.enter_context(tc.tile_pool(name="const", bufs=1))
        xt_pool = ctx.enter_context(tc.tile_pool(name="xt", bufs=1))
        wn_pool = ctx.enter_context(tc.tile_pool(name="wn", bufs=2))
        wt_pool = ctx.enter_context(tc.tile_pool(name="wt", bufs=WT_BUFS))
        y_pool = ctx.enter_context(tc.tile_pool(name="y", bufs=3))
        ps_pool = ctx.enter_context(tc.tile_pool(name="ps", bufs=4, space="PSUM"))
        pt_pool = ctx.enter_context(tc.tile_pool(name="pt", bufs=4, space="PSUM"))

        # ---- gpsimd cast DMAs (f32 -> bf16) ----
        nc.gpsimd.dma_start(cbrep_h[0:MOD], cb_h[:])
        for bt in range(NB // 128):
            nc.gpsimd.dma_start(
                xb_h[bt * 128:(bt + 1) * 128, :].flatten(),
                x_h[bt * 128:(bt + 1) * 128, :].flatten(),
            )
        bias16 = const_pool.tile([1, OUT_DIM], BF16)
        nc.gpsimd.dma_start(bias16[:], bias_h[:])       # cast f32 -> bf16

        # ---- constants ----
        ident_sb = const_pool.tile([128, 128], BF16)
        nc.sync.dma_start(ident_sb[:], ident_h[:, :])
        ones_sb = const_pool.tile([1, 128], BF16)
        nc.sync.dma_start(ones_sb[:], ones_h[:, :])

        # ---- x^T via xbar transposes (single HWDGE master: nc.sync) ----
        xT = xt_pool.tile([128, KT * NB], BF16)
        def emit_xt():
            for bt in range(NB // 128):
                for kt in range(KT):
                    nc.sync.dma_start(
                        xT[:, kt * NB + bt * 128: kt * NB + (bt + 1) * 128],
                        xb_h[bt * 128:(bt + 1) * 128, kt * 128:(kt + 1) * 128],
                        transpose=True,
                    )

        # ---- table build (log-doubling replication: 7 copies) ----
        n = MOD
        while n < CBREP:
            m = min(n, CBREP - n)
            nc.sync.dma_start(cbrep_h[n:n + m], cbrep_h[0:m])
            n += m

        T1 = build_pool.tile([128, 512], BF16, tag="T1")
        nc.sync.dma_start(
            T1[:],
            cbrep_h[0:128 * HB].rearrange("(p s) -> p s", s=HB)[:, 0:512],
        )
        T2 = build_pool.tile([16, 512], BF16, tag="T2")
        b2 = 115 * HB
        nc.sync.dma_start(
            T2[:],
            cbrep_h[b2:b2 + 16 * HB].rearrange("(p s) -> p s", s=HB)[:, 0:512],
        )
        eps1_sb = build_pool.tile([128, 512], BF16, tag="e1")
        nc.sync.dma_start(eps1_sb[:], eps1_h[:, :])
        eps2_sb = build_pool.tile([16, 512], BF16, tag="e2")
        nc.sync.dma_start(eps2_sb[:], eps2_h[:, :])
        V1 = build_pool.tile([128, 512], BF16, tag="V1")
        nc.vector.tensor_mul(V1[:], T1[:], eps1_sb[:])
        V2 = build_pool.tile([16, 512], BF16, tag="V2")
        nc.vector.tensor_mul(V2[:], T2[:], eps2_sb[:])

        for c in range(4):
            U1 = build_pool.tile([128, 128], BF16, tag="U1")
            nc.sync.dma_start(U1[:], V1[:, 128 * c:128 * (c + 1)], transpose=True)
            off = 131 * 128 * c
            nc.sync.dma_start(
                p1_h[off:off + 131 * 128].rearrange("(p s) -> p s", s=131)[:, 0:128],
                U1[:],
            )
            U2 = build_pool.tile([128, 16], BF16, tag="U2")
            nc.sync.dma_start(U2[:], V2[:, 128 * c:128 * (c + 1)], transpose=True)
            off2 = 115 + 131 * 128 * c
            nc.sync.dma_start(
                p1_h[off2:off2 + 131 * 128].rearrange("(p s) -> p s", s=131)[:, 0:16],
                U2[:],
            )
        # tail replicate (positions >= 65536 mirror j - 65536)
        nc.sync.dma_start(p1_h[MOD:TBL], p1_h[0:TBL - MOD])
        emit_xt()

        # ---- main loop ----
        for oc in range(NOC):
            # W^T tiles for this o-chunk: load W-natural windows (one 1 MB DMA
            # per 128 o's), flip on the PE, drain into wt via DVE/ACT copies.
            wt = []
            for _kt in range(KT):
                wh = []
                for _h in range(2):
                    w = wt_pool.tile([128, 512], BF16, tag="wt",
                                     name=f"wt{_kt}_{_h}")
                    wh.append(w)
                wt.append(wh)
            for osub in range(OC // 128):
                o0 = oc * OC + osub * 128
                base = (T0 + S * o0) % MOD
                # kt in [0, KTX): xbar transpose straight into wt
                for kt in range(KTX):
                    win = p1_h[base + kt * 128:base + kt * 128 + S * 128].rearrange(
                        "(p s) -> p s", s=S
                    )[:, 0:128]
                    nc.sync.dma_start(
                        wt[kt][osub // 4][:, (osub % 4) * 128:(osub % 4 + 1) * 128],
                        win, transpose=True
                    )
                # kt in [KTX, KT): PE transpose from a W-natural window
                wn = wn_pool.tile([128, WNF], BF16, tag="wn")
                nc.sync.dma_start(wn[:], _win_ap(p1_h, base + KTX * 128))
                for kt in range(KTX, KT):
                    pst = pt_pool.tile([128, 128], BF16, tag="pt")
                    nc.tensor.transpose(
                        pst[:], wn[:, (kt - KTX) * 128:(kt - KTX + 1) * 128],
                        ident_sb[:],
                    )
                    dst = wt[kt][osub // 4][:, (osub % 4) * 128:(osub % 4 + 1) * 128]
                    if kt % 2 == 0:
                        nc.vector.tensor_copy(dst, pst[:])
                    else:
                        nc.scalar.copy(dst, pst[:])
            for bt in range(NB // 128):
                ps0 = ps_pool.tile([128, 512], F32, tag="ps")
                ps1 = ps_pool.tile([128, 512], F32, tag="ps")
                for kt in range(KT):
                    xoff = kt * NB + bt * 128
                    lhsT = xT[:, xoff:xoff + 128]
                    nc.tensor.matmul(
                        ps0[:], lhsT, wt[kt][0][:],
                        start=(kt == 0), stop=False,
                    )
                    nc.tensor.matmul(
                        ps1[:], lhsT, wt[kt][1][:],
                        start=(kt == 0), stop=False,
                    )
                for h, ps in ((0, ps0), (1, ps1)):
                    ob = oc * OC + h * 512
                    # bias via K=1 accumulation (closes the psum group)
                    nc.tensor.matmul(
                        ps[:], ones_sb[:], bias16[0:1, ob:ob + 512],
                        start=False, stop=True,
                    )
                    yt = y_pool.tile([128, 512], F32, tag="y")
                    nc.vector.tensor_copy(yt[:], ps[:])
                    nc.sync.dma_start(
                        out_h[bt * 128:(bt + 1) * 128, ob:ob + 512], yt[:]
                    )


# This container's walrus rejects the EVENT_SEMAPHORE_RANGE_CLEAR ISA encoding
# ("ISA wrong length") that TileContext emits when freeing semaphores at kernel
# exit. The preamble zeroes all semaphore banks via InstMemset at the start of
# every execution, so the exit-time clear is redundant — skip emitting it but
# keep the allocator bookkeeping.
def _patched_clear_and_free_semaphores(self, sems):
    if not sems:
        return
    sem_nums = [
        sem.num if isinstance(sem, bass.SemaphoreHandle) else sem for sem in sems
    ]
    self._state.prepend_free_semaphores(sem_nums)
    for poison_set in self._tile_sem_poison_stack:
        poison_set.update(sem_nums)


bass.Bass.clear_and_free_semaphores = _patched_clear_and_free_semaphores


# Same walrus also only encodes ONE sync-wait on non-EventSemaphore
# instructions ("Too many sync wait commands"), but TileContext's kernel-tail
# drain gets one wait per outstanding DMA sem lane. Split the extras onto
# additional drains (sequential on the same engine => same semantics).
def _patched_drain_and_barrier(self, tick_clock, wait_clock):
    import bass_rust as _br
    from concourse.vector_clock import ScopedClock

    nc = self.nc
    drain_inst = nc.sync.drain()
    wait_clock.add_sem_waits(
        drain_inst.ins, ScopedClock({None: tick_clock.global_clock})
    )
    si = drain_inst.ins.sync_info
    if si is not None and si.on_wait and len(si.on_wait) > 1:
        waits = list(si.on_wait)
        si.on_wait = waits[:1]
        for w in waits[1:]:
            d2 = nc.sync.drain()
            d2.ins.sync_info = _br.SyncInfo(on_wait=[w], on_update=[])
    nc.all_engine_barrier()
    assert self.sems is not None
    popped = nc._tile_sem_poison_stack.pop()
    assert popped is self._sem_poison
    nc.clear_and_free_semaphores(list(self.sems.allocated().values()))
    nc.all_engine_barrier()


tile.TileContext._drain_and_barrier = _patched_drain_and_barrier


def _split_multiwait(nc):
    """Walrus in this container encodes at most 1 sync-wait per instruction
    (2 for EventSemaphore). Tile's scheduler attaches more. Move extra waits
    onto InstNoOp carriers inserted just before the instruction in its block
    (same engine => executes in order => identical semantics)."""
    import bass_rust as _br

    for f in nc.m.functions:
        for blk in f.blocks:
            insts = blk.instructions
            i = 0
            while i < len(insts):
                inst = insts[i]
                si = getattr(inst, "sync_info", None)
                cap = 2 if type(inst).__name__ == "InstEventSemaphore" else 1
                if si is not None and si.on_wait and len(si.on_wait) > cap:
                    waits = list(si.on_wait)
                    si.on_wait = waits[:cap]
                    for w in waits[cap:]:
                        nop = nc.engines[inst.engine].nop()
                        nopi = nop.ins
                        nopi.sync_info = _br.SyncInfo(on_wait=[w], on_update=[])
                        src_list = nc.cur_bb.bb.instructions
                        assert src_list[len(src_list) - 1].name == nopi.name
                        src_list.pop()
                        insts.insert(i, nopi)
                        i += 1
                i += 1


def _dedup_ldweights(nc):
    """bass lowers every matmul to Ldweights+Matmult; consecutive matmuls that
    share the stationary operand reload it redundantly (walrus --enable-ldw-opt
    is off in this harness). Drop an InstLdweights when the previous PE
    Ldweights in the block has an identical weights AP and only Matmult/NoOp
    sit between (they don't clobber the array). LDWs carry no sync_info here
    (waits ride on the Matmults), so deletion is semaphore-safe."""
    n_del = 0
    for f in nc.m.functions:
        for blk in f.blocks:
            insts = blk.instructions
            last_key = None
            i = 0
            while i < len(insts):
                inst = insts[i]
                tn = type(inst).__name__
                eng = getattr(inst, "engine", None)
                if eng is not None and str(eng) == "EngineType.PE":
                    if tn == "InstLdweights":
                        si = getattr(inst, "sync_info", None)
                        key = repr(inst.ins[0])
                        if (key == last_key and
                                (si is None or not si.on_wait)):
                            insts.pop(i)
                            n_del += 1
                            continue
                        last_key = key
                    elif tn not in ("InstMatmult", "InstNoOp"):
                        last_key = None
                i += 1
    return n_del


_NC_CACHE = None


def _build_nc():
    global _NC_CACHE
    if _NC_CACHE is not None:
        return _NC_CACHE
    nc = bass.Bass(trn_type="TRN2")
    x_h = nc.dram_tensor("x", [NB, IN_DIM], F32, kind="ExternalInput")
    cb_h = nc.dram_tensor("codebook", [MOD], F32, kind="ExternalInput")
    bias_h = nc.dram_tensor("bias", [OUT_DIM], F32, kind="ExternalInput")
    out_h = nc.dram_tensor("out", [NB, OUT_DIM], F32, kind="ExternalOutput")
    with tile.TileContext(nc) as tc:
        build_kernel(tc, out_h, x_h, cb_h, bias_h)
    _split_multiwait(nc)
    _dedup_ldweights(nc)
    _NC_CACHE = nc
    return nc


def kernel(x, codebook, bias):
    x = np.ascontiguousarray(np.asarray(x, dtype=np.float32))
    codebook = np.ascontiguousarray(np.asarray(codebook, dtype=np.float32))
    bias = np.ascontiguousarray(np.asarray(bias, dtype=np.float32))
    assert x.shape == (8192, 4096)

    from concourse.bass_utils import run_bass_kernel_spmd

    nc = _build_nc()
    in_maps = [
        {"x": x[c * NB:(c + 1) * NB], "codebook": codebook, "bias": bias}
        for c in range(8)
    ]
    trace = os.environ.get("KERNEL_TRACE", "0") == "1"
    res = run_bass_kernel_spmd(nc, in_maps, core_ids=list(range(8)), trace=trace)
    if trace and res.exec_time_ns is not None:
        print(f"HW exec time: {res.exec_time_ns} ns")
    out = np.concatenate([r["out"] for r in res.results], axis=0)
    return out
